# revision 6
# baseline (speedup 1.0000x reference)
"""Bass/Trainium2 kernel for nn_BinaryResNetBlock (bireal block, stride 1).

Computation (reference):
    stage(x, W, g, b): a = sign(x); wb = mean(|W|)*sign(W)
                       y = conv3x3(a, wb, pad=1); BN(train-mode, batch stats)
    inner = stage(x, W1, g1, b1) + x
    out   = stage(inner, W2, g2, b2) + inner

Strategy:
  - Data parallel over batch: N=32 -> 4 images per core on 8 cores.
  - conv(sign(x), sign(W)) accumulates exact small integers in fp32 PSUM, so
    fp8(e4m3) matmuls in DoubleRow mode (K=256 per MM) are bit-exact.
    Conv outputs stored as int16 (|c| <= 2304).
  - PW=57 shared-pad layout: the right pad of row r is row r+1's left pad, so
    each row carries one junk column instead of two (matmul free dim 456).
  - Convs ordered ch_o-outer: all images' out-chunk-0 convs first, then
    chunk 1.  BN batch stats are all-reduced per out-chunk (1KB payloads):
    the chunk-0 collective launches at mid-conv and is fully hidden; only
    the chunk-1 collective's latency is exposed per stage.
  - Stage-2 sign planes are double-buffered (a2) so the chunk-0 re-sign
    (t = c1*A1 + x; a2 = Sign(t + B1)) runs during conv1's chunk-1 work and
    the collective flight.  Final chunk-0 (u = c2*A2 + B12; out = u + t;
    writeback) hides under conv2's chunk-1 work; only final chunk-1 is an
    exposed tail.
  - Engine budget: ACT does signs + chunk-0 PSUM drains + final scale ops;
    DVE does bn_stats + chunk-1 drains + t/final adds; GpSimd does casts,
    borders, collective staging; sync/gpsimd DMA queues split ch0/ch1.
"""

import os
import sys

import numpy as np


def _ensure_path():
    try:
        import concourse.bass  # noqa: F401
    except ImportError:
        for p in ("/opt/trn_rl_repo", "/root/.axon_site/_ro/trn_rl_repo"):
            if os.path.isdir(p) and p not in sys.path:
                sys.path.insert(0, p)


_ensure_path()

import ml_dtypes  # noqa: E402

import concourse.bacc as bacc  # noqa: E402
import concourse.bass as bass  # noqa: E402
import concourse.mybir as mybir  # noqa: E402
import concourse.tile as tile  # noqa: E402
from concourse import bass_utils  # noqa: E402

F32 = mybir.dt.float32
I16 = mybir.dt.int16
F8 = mybir.dt.float8e4
F16 = mybir.dt.float16
NP_F8 = ml_dtypes.float8_e4m3

C = 256  # channels
P = 128  # partitions
NCH = C // P  # channel chunks (2)
WID = 56  # image width (fixed)
PW = WID + 1  # padded width (57): right pad shared with next row's left pad
RB = 8  # output rows per PSUM tile
EPS = 1e-5

# module-level knobs (test.py may set these)
TRACE = False
TRACE_KW = {}

Alu = mybir.AluOpType
Act = mybir.ActivationFunctionType
DR = mybir.MatmulPerfMode.DoubleRow


def build_nc(n_img, h, n_cores):
    """Build the SPMD Bass program (same on every core)."""
    assert h % RB == 0 and h % 4 == 0
    nrb = h // RB
    ph = h + 2
    plane = ph * PW
    pstride = (plane + 15) // 16 * 16  # DoubleRow needs 16B-aligned k-step
    hw = h * WID
    FREE = RB * PW  # matmul free dim (456); col w=56 of each row is junk
    hrows = h // 2
    half = hrows * WID
    qrows = h // 4
    qsz = qrows * WID
    m_loc = n_img * hw
    m_glob = n_cores * m_loc

    nc = bacc.Bacc(
        "TRN2", target_bir_lowering=False, debug=False, num_devices=n_cores
    )
    x_d = nc.dram_tensor("x", [n_img, C, h, WID], F32, kind="ExternalInput").ap()
    w_d = [
        nc.dram_tensor(f"wb{s + 1}", [P, 9, NCH, C], F8, kind="ExternalInput").ap()
        for s in range(2)
    ]
    # coefs[:, ch, k]: k=0 gamma1*scale1, 1 beta1, 2 gamma2*scale2, 3 beta2,
    #                 4 scale1^2 (bcast), 5 scale2^2 (bcast)
    cf_d = nc.dram_tensor("coefs", [P, NCH, 6], F32, kind="ExternalInput").ap()
    out_d = nc.dram_tensor("out", [n_img, C, h, WID], F32, kind="ExternalOutput").ap()

    with tile.TileContext(nc) as tc:
        with (
            tc.tile_pool(name="persist", bufs=1) as persist,
            tc.tile_pool(name="abuf", bufs=1) as abuf,
            tc.tile_pool(name="cbuf", bufs=1) as cbuf,
            tc.tile_pool(name="statsp", bufs=1) as statsp,
            tc.tile_pool(name="xs", bufs=4) as xs,
            tc.tile_pool(name="ftp", bufs=2) as ftp,
            tc.tile_pool(name="small", bufs=2) as small,
            tc.tile_pool(name="ps", bufs=8, space="PSUM") as psp,
            tc.tile_pool(name="dram", bufs=2, space="DRAM") as dramp,
        ):
            # ---- persistent tiles; boot DMAs ----
            w_t = [
                persist.tile([P, 9, NCH, C], F8, tag=f"w{s}", name=f"w{s}")
                for s in range(2)
            ]
            nc.sync.dma_start(out=w_t[0], in_=w_d[0])
            coefs = persist.tile([P, NCH, 6], F32, tag="coefs")
            nc.sync.dma_start(out=coefs, in_=cf_d)
            nc.scalar.dma_start(out=w_t[1], in_=w_d[1])  # spare queue; not urgent
            eps_t = persist.tile([P, 1], F32, tag="eps")
            nc.vector.memset(eps_t, EPS)
            stt1 = persist.tile([P, NCH, 6], F32, tag="stt1")
            stt2 = persist.tile([P, NCH, 6], F32, tag="stt2")
            b12 = persist.tile([P, NCH, 1], F32, tag="b12")

            # prewarm ACT tables (Sign now; Sqrt/Identity used post-collective)
            warm = small.tile([P, 1], F32, tag="warm")
            nc.scalar.activation(out=warm, in_=eps_t, func=Act.Sign)
            nc.scalar.activation(out=warm, in_=eps_t, func=Act.Sqrt)
            nc.scalar.activation(out=warm, in_=eps_t, func=Act.Identity,
                                 bias=eps_t)

            c_t = [
                {
                    (i, ch): cbuf.tile(
                        [P, hw], I16, tag=f"c{s}_{i}_{ch}", name=f"c{s}_{i}_{ch}"
                    )
                    for i in range(n_img)
                    for ch in range(NCH)
                }
                for s in range(2)
            ]
            stats1 = statsp.tile([P, NCH, n_img * nrb, 6], F32, tag="stats1")
            stats2 = statsp.tile([P, NCH, n_img * nrb, 6], F32, tag="stats2")

            a1 = [
                abuf.tile([P, NCH, pstride], F8, tag=f"a1_{i}", name=f"a1_{i}")
                for i in range(n_img)
            ]
            a2 = [
                abuf.tile([P, NCH, pstride], F8, tag=f"a2_{i}", name=f"a2_{i}")
                for i in range(n_img)
            ]

            def zero_borders(eng, a_t, ch):
                """Zero pad rows/col + tail of one padded plane ([ph, PW])."""
                pl = a_t[:, ch, 0:plane].rearrange("p (r c) -> p r c", c=PW)
                eng.memset(pl[:, 0:1, :], 0.0)  # top pad row
                eng.memset(pl[:, h + 1 : h + 2, :], 0.0)  # bottom pad row
                eng.memset(pl[:, 1 : h + 1, 0:1], 0.0)  # left pad col
                if pstride > plane:
                    eng.memset(a_t[:, ch, plane:pstride], 0.0)

            def sign_view(a_t, ch):
                """Interior [P, h, WID] view of the padded plane."""
                return a_t[:, ch, 0:plane].rearrange(
                    "p (r c) -> p r c", c=PW
                )[:, 1 : h + 1, 1 : WID + 1]

            # a1[0] borders on DVE (idle at boot); everything else on GpSimd
            for ch in range(NCH):
                zero_borders(nc.vector, a1[0], ch)

            # prewarm the collective path (first collective pays ~60us setup)
            wc_in = dramp.tile([P, 1], F32, tag="wc_in", name="wc_in")
            wc_out = dramp.tile(
                [n_cores * P, 1], F32, tag="wc_out", name="wc_out",
                addr_space="Shared" if n_cores > 4 else "Local",
            )
            nc.gpsimd.dma_start(out=wc_in, in_=cf_d[:, 0, 0:1])
            nc.gpsimd.collective_compute(
                "AllGather",
                Alu.bypass,
                replica_groups=[list(range(n_cores))],
                ins=[wc_in.opt()],
                outs=[wc_out.opt()],
            )

            # ---- stage-1 x loads (halves, dual queues) + signs (quarters) ----
            for i in range(n_img):
                xts = {}
                for hh in range(2):
                    for ch in range(NCH):
                        xt = xs.tile([P, half], F32, tag="x", name=f"x1_{i}")
                        eng = nc.sync if ch == 0 else nc.gpsimd
                        eng.dma_start(
                            out=xt.rearrange("p (r c) -> p r c", c=WID),
                            in_=x_d[i, ch * P : (ch + 1) * P,
                                    hh * hrows : (hh + 1) * hrows],
                        )
                        xts[(ch, hh)] = xt
                for q in range(4):
                    for ch in range(NCH):
                        xt = xts[(ch, q // 2)]
                        sl = slice((q % 2) * qsz, (q % 2 + 1) * qsz)
                        nc.scalar.activation(
                            out=sign_view(a1[i], ch)[
                                :, q * qrows : (q + 1) * qrows, :
                            ],
                            in_=xt[:, sl].rearrange("p (r c) -> p r c", c=WID),
                            func=Act.Sign,
                        )
                # remaining borders on gpsimd, pipelined behind img i's loads
                if i + 1 < n_img:
                    for ch in range(NCH):
                        zero_borders(nc.gpsimd, a1[i + 1], ch)
                for ch in range(NCH):
                    zero_borders(nc.gpsimd, a2[i], ch)

            def conv_chunk(s, wt, a_t, i, ch_o, stats_t, drain_dve):
                """3x3 binary conv, one image, one out-chunk; drain + stats.

                drain_dve: PSUM->int16 copy on DVE instead of ACT (used for
                chunk-1 convs so ACT stays free for collective-gated signs).
                """
                c_tile = c_t[s][(i, ch_o)]
                for rb in range(nrb):
                    ps = psp.tile([P, FREE], F32, tag="ps", name="ps")
                    for tap in range(9):
                        dh, dw = divmod(tap, 3)
                        off = (rb * RB + dh) * PW + dw
                        nc.tensor.matmul(
                            ps,
                            wt[:, tap, :, ch_o * P : (ch_o + 1) * P],
                            a_t[:, 0:2, off : off + FREE],
                            start=(tap == 0),
                            stop=(tap == 8),
                            perf_mode=DR,
                        )
                    pv = ps.rearrange("p (r c) -> p r c", c=PW)[:, :, 0:WID]
                    cs = c_tile[:, rb * RB * WID : (rb + 1) * RB * WID]
                    csv = cs.rearrange("p (r c) -> p r c", c=WID)
                    if drain_dve:
                        nc.vector.tensor_copy(out=csv, in_=pv)
                    else:
                        nc.scalar.copy(out=csv, in_=pv)
                    nc.vector.bn_stats(out=stats_t[:, ch_o, i * nrb + rb], in_=cs)

            def ar_launch(stats_t, ch):
                """Aggregate local stats for one chunk, launch AllGather."""
                mv = small.tile([P, 2], F32, tag="mv")
                nc.vector.bn_aggr(out=mv, in_=stats_t[:, ch])
                ar = small.tile([P, 2], F32, tag="ar")
                tmp = small.tile([P, 1], F32, tag="tmp")
                # S1 = mean * m_loc ; S2 = (var + mean^2) * m_loc
                nc.vector.tensor_scalar(
                    out=ar[:, 0:1], in0=mv[:, 0:1],
                    scalar1=float(m_loc), scalar2=None, op0=Alu.mult,
                )
                nc.vector.tensor_mul(tmp, mv[:, 0:1], mv[:, 0:1])
                nc.vector.tensor_add(tmp, tmp, mv[:, 1:2])
                nc.vector.tensor_scalar(
                    out=ar[:, 1:2], in0=tmp,
                    scalar1=float(m_loc), scalar2=None, op0=Alu.mult,
                )
                d_in = dramp.tile([P, 2], F32, tag="d_in")
                d_out = dramp.tile(
                    [n_cores * P, 2], F32, tag="d_out",
                    addr_space="Shared" if n_cores > 4 else "Local",
                )
                nc.gpsimd.dma_start(out=d_in, in_=ar)
                nc.gpsimd.collective_compute(
                    "AllGather",
                    Alu.bypass,
                    replica_groups=[list(range(n_cores))],
                    ins=[d_in.opt()],
                    outs=[d_out.opt()],
                )
                return d_out

            def ar_finish(d_out, ch, gs_col, b_col, ssq_col, stt):
                """Fetch gathered stats, compute per-channel A, B for chunk.

                stt columns: 0 mu_c, 1 var_c, 2 inv, 3 A, 4 B, 5 tmp
                """
                g_all = small.tile([P, n_cores, 2], F32, tag="g_all")
                nc.gpsimd.dma_start(
                    out=g_all, in_=d_out.rearrange("(r p) f -> p r f", p=P)
                )
                g = small.tile([P, 2], F32, tag="g")
                nc.vector.tensor_reduce(
                    out=g,
                    in_=g_all.rearrange("p r f -> p f r"),
                    axis=mybir.AxisListType.X,
                    op=Alu.add,
                )
                inv_m = float(1.0 / m_glob)
                s = stt[:, ch]
                nc.vector.tensor_scalar(
                    out=s[:, 0:1], in0=g[:, 0:1],
                    scalar1=inv_m, scalar2=None, op0=Alu.mult,
                )
                nc.vector.tensor_scalar(
                    out=s[:, 1:2], in0=g[:, 1:2],
                    scalar1=inv_m, scalar2=None, op0=Alu.mult,
                )
                nc.vector.tensor_mul(s[:, 5:6], s[:, 0:1], s[:, 0:1])
                nc.vector.tensor_sub(s[:, 1:2], s[:, 1:2], s[:, 5:6])
                # sd = sqrt(var_c * scale^2 + eps); inv = 1/sd
                nc.scalar.activation(
                    out=s[:, 2:3], in_=s[:, 1:2], func=Act.Sqrt,
                    bias=eps_t, scale=coefs[:, 0, ssq_col : ssq_col + 1],
                )
                nc.vector.reciprocal(out=s[:, 2:3], in_=s[:, 2:3])
                # A = inv * (gamma*scale);  B = beta - mu_c * A
                nc.vector.tensor_mul(
                    s[:, 3:4], s[:, 2:3], coefs[:, ch, gs_col : gs_col + 1]
                )
                nc.vector.tensor_mul(s[:, 5:6], s[:, 0:1], s[:, 3:4])
                nc.vector.tensor_sub(
                    s[:, 4:5], coefs[:, ch, b_col : b_col + 1], s[:, 5:6]
                )

            inner_t = {}

            def resign_img(i, ch):
                """Stage-2 input prep for one (image, chunk):
                t = c1*A1 + x; inner = t (fp16, reusing the c1 slot);
                a2 = Sign(t + B1)."""
                c1 = c_t[0][(i, ch)]
                xts = []
                for hh in range(2):
                    xt = xs.tile([P, half], F32, tag="x", name=f"x2_{i}_{ch}")
                    nc.sync.dma_start(
                        out=xt.rearrange("p (r c) -> p r c", c=WID),
                        in_=x_d[i, ch * P : (ch + 1) * P,
                                hh * hrows : (hh + 1) * hrows],
                    )
                    nc.vector.scalar_tensor_tensor(
                        out=xt,
                        in0=c1[:, hh * half : (hh + 1) * half],
                        scalar=stt1[:, ch, 3:4],
                        in1=xt,
                        op0=Alu.mult,
                        op1=Alu.add,
                    )
                    xts.append(xt)
                it = cbuf.tile(
                    [P, hw], F16, tag=f"c0_{i}_{ch}", name=f"inner_{i}_{ch}"
                )
                for hh in range(2):
                    nc.gpsimd.tensor_copy(
                        out=it[:, hh * half : (hh + 1) * half], in_=xts[hh]
                    )
                for q in range(4):
                    sl = slice((q % 2) * qsz, (q % 2 + 1) * qsz)
                    nc.scalar.activation(
                        out=sign_view(a2[i], ch)[:, q * qrows : (q + 1) * qrows, :],
                        in_=xts[q // 2][:, sl].rearrange("p (r c) -> p r c", c=WID),
                        func=Act.Sign,
                        bias=stt1[:, ch, 4:5],
                    )
                inner_t[(i, ch)] = it

            def final_img(i, ch, out_eng):
                """u = c2*A2 + (B1+B2) on ACT; out = u + t on DVE; write out."""
                c2 = c_t[1][(i, ch)]
                for hh in range(2):
                    sl = slice(hh * half, (hh + 1) * half)
                    ft = ftp.tile([P, half], F16, tag="f16", name="ft")
                    nc.scalar.activation(
                        out=ft, in_=c2[:, sl],
                        func=Act.Identity,
                        bias=b12[:, ch], scale=stt2[:, ch, 3:4],
                    )
                    fo = xs.tile([P, half], F32, tag="x", name="fout")
                    nc.vector.tensor_add(fo, ft, inner_t[(i, ch)][:, sl])
                    out_eng.dma_start(
                        out=out_d[i, ch * P : (ch + 1) * P,
                                  hh * hrows : (hh + 1) * hrows],
                        in_=fo.rearrange("p (r c) -> p r c", c=WID),
                    )

            # ================= stage 1 =================
            with nc.named_scope("conv1a"):
                for i in range(n_img):
                    conv_chunk(0, w_t[0], a1[i], i, 0, stats1, drain_dve=False)
            with nc.named_scope("ar1a"):
                d1a = ar_launch(stats1, 0)
            # conv1 chunk 1 interleaved with chunk-0 re-sign.  ar_finish (and
            # the collective-gated t-computes behind it) is emitted after two
            # images of chunk-1 drains so the DVE FIFO never head-of-line
            # blocks the PSUM drains the PE needs for bank reuse.
            with nc.named_scope("conv1b"):
                conv_chunk(0, w_t[0], a1[0], 0, 1, stats1, drain_dve=True)
                conv_chunk(0, w_t[0], a1[1], 1, 1, stats1, drain_dve=True)
                ar_finish(d1a, 0, 0, 1, 4, stt1)
                resign_img(0, 0)
                conv_chunk(0, w_t[0], a1[2], 2, 1, stats1, drain_dve=True)
                resign_img(1, 0)
                conv_chunk(0, w_t[0], a1[3], 3, 1, stats1, drain_dve=True)
                resign_img(2, 0)
                resign_img(3, 0)
            with nc.named_scope("ar1b"):
                d1b = ar_launch(stats1, 1)
                ar_finish(d1b, 1, 0, 1, 4, stt1)

            # ================= stage 2 =================
            with nc.named_scope("conv2a"):
                resign_img(0, 1)
                conv_chunk(1, w_t[1], a2[0], 0, 0, stats2, drain_dve=False)
                resign_img(1, 1)
                conv_chunk(1, w_t[1], a2[1], 1, 0, stats2, drain_dve=False)
                resign_img(2, 1)
                conv_chunk(1, w_t[1], a2[2], 2, 0, stats2, drain_dve=False)
                resign_img(3, 1)
                conv_chunk(1, w_t[1], a2[3], 3, 0, stats2, drain_dve=False)
            with nc.named_scope("ar2a"):
                d2a = ar_launch(stats2, 0)
            with nc.named_scope("conv2b"):
                conv_chunk(1, w_t[1], a2[0], 0, 1, stats2, drain_dve=True)
                conv_chunk(1, w_t[1], a2[1], 1, 1, stats2, drain_dve=True)
                ar_finish(d2a, 0, 2, 3, 5, stt2)
                nc.vector.tensor_add(
                    b12[:, 0], stt1[:, 0, 4:5], stt2[:, 0, 4:5]
                )
                final_img(0, 0, nc.sync)
                conv_chunk(1, w_t[1], a2[2], 2, 1, stats2, drain_dve=True)
                final_img(1, 0, nc.sync)
                conv_chunk(1, w_t[1], a2[3], 3, 1, stats2, drain_dve=True)
                final_img(2, 0, nc.sync)
                final_img(3, 0, nc.sync)
            with nc.named_scope("ar2b"):
                d2b = ar_launch(stats2, 1)
                ar_finish(d2b, 1, 2, 3, 5, stt2)
                nc.vector.tensor_add(
                    b12[:, 1], stt1[:, 1, 4:5], stt2[:, 1, 4:5]
                )
            with nc.named_scope("final"):
                for i in range(n_img):
                    final_img(i, 1, nc.gpsimd)
    return nc


def prep_inputs(x, W1, gamma1, beta1, W2, gamma2, beta2, n_cores, n_img):
    """Host-side prep: shard x, binarize/permute weights, pack BN coefs."""

    def prep_w(Wm):
        Wm = np.asarray(Wm, np.float32)
        scale = np.float32(np.mean(np.abs(Wm)))
        s = np.sign(Wm).astype(NP_F8)  # [co, ci, 3, 3]
        t = s.reshape(C, NCH, P, 3, 3)  # co, kch, p, dh, dw
        t = np.ascontiguousarray(t.transpose(2, 3, 4, 1, 0))  # p,dh,dw,kch,co
        return t.reshape(P, 9, NCH, C), scale

    w1b, s1 = prep_w(W1)
    w2b, s2 = prep_w(W2)
    g1 = np.asarray(gamma1, np.float32)
    b1 = np.asarray(beta1, np.float32)
    g2 = np.asarray(gamma2, np.float32)
    b2 = np.asarray(beta2, np.float32)
    coefs = np.zeros((P, NCH, 6), np.float32)
    coefs[:, :, 0] = (g1 * s1).reshape(NCH, P).T
    coefs[:, :, 1] = b1.reshape(NCH, P).T
    coefs[:, :, 2] = (g2 * s2).reshape(NCH, P).T
    coefs[:, :, 3] = b2.reshape(NCH, P).T
    coefs[:, :, 4] = np.float32(s1) ** 2
    coefs[:, :, 5] = np.float32(s2) ** 2

    x = np.asarray(x, np.float32)
    n, _, h, _ = x.shape
    assert n == n_cores * n_img
    xs = x.reshape(n_cores, n_img, C, h, WID)
    return [
        {
            "x": np.ascontiguousarray(xs[c]),
            "wb1": w1b,
            "wb2": w2b,
            "coefs": coefs,
        }
        for c in range(n_cores)
    ]


_NC_CACHE = {}


def _get_nc(n_img, h, n_cores):
    key = (n_img, h, n_cores)
    if key not in _NC_CACHE:
        nc = build_nc(n_img, h, n_cores)
        nc.compile()
        _NC_CACHE[key] = nc
    return _NC_CACHE[key]


_LAST_RESULT = None  # BassKernelResults of the most recent run (for test.py)


def kernel(x, W1, gamma1, beta1, W2, gamma2, beta2):
    global _LAST_RESULT
    x = np.asarray(x, np.float32)
    n_cores = 8
    n = x.shape[0]
    assert n % n_cores == 0
    n_img = n // n_cores
    h = x.shape[2]

    nc = _get_nc(n_img, h, n_cores)
    in_maps = prep_inputs(
        x, W1, gamma1, beta1, W2, gamma2, beta2, n_cores, n_img
    )
    res = bass_utils.run_bass_kernel_spmd(
        nc, in_maps, core_ids=list(range(n_cores)), trace=TRACE, **TRACE_KW
    )
    _LAST_RESULT = res
    out = np.concatenate([res.results[c]["out"] for c in range(n_cores)], axis=0)
    return out


# revision 10
# speedup vs baseline: 1.0766x; 1.0766x over previous
"""Bass/Trainium2 kernel for nn_BinaryResNetBlock (bireal block, stride 1).

Computation (reference):
    stage(x, W, g, b): a = sign(x); wb = mean(|W|)*sign(W)
                       y = conv3x3(a, wb, pad=1); BN(train-mode, batch stats)
    inner = stage(x, W1, g1, b1) + x
    out   = stage(inner, W2, g2, b2) + inner

Strategy:
  - Data parallel over batch: N=32 -> 4 images per core on 8 cores.
  - conv(sign(x), sign(W)) accumulates exact small integers in fp32 PSUM, so
    fp8(e4m3) matmuls in DoubleRow mode (K=256 per MM) are bit-exact.
    Conv outputs stored as int16 (|c| <= 2304).
  - PW=57 shared-pad layout: the right pad of row r is row r+1's left pad, so
    each row carries one junk column instead of two (matmul free dim 456).
  - Convs ordered ch_o-outer: all images' out-chunk-0 convs first, then
    chunk 1.  BN batch stats are all-reduced per out-chunk (1KB payloads):
    the chunk-0 collective launches at mid-conv and is fully hidden; only
    the chunk-1 collective's latency is exposed per stage.
  - Stage-2 sign planes are double-buffered (a2) so the chunk-0 re-sign
    (t = c1*A1 + x; a2 = Sign(t + B1)) runs during conv1's chunk-1 work and
    the collective flight.  Final chunk-0 (u = c2*A2 + B12; out = u + t;
    writeback) hides under conv2's chunk-1 work; only final chunk-1 is an
    exposed tail.
  - Engine budget: ACT does signs + chunk-0 PSUM drains + final scale ops;
    DVE does bn_stats + chunk-1 drains + t/final adds; GpSimd does casts,
    borders, collective staging; sync/gpsimd DMA queues split ch0/ch1.
"""

import os
import sys

import numpy as np


def _ensure_path():
    try:
        import concourse.bass  # noqa: F401
    except ImportError:
        for p in ("/opt/trn_rl_repo", "/root/.axon_site/_ro/trn_rl_repo"):
            if os.path.isdir(p) and p not in sys.path:
                sys.path.insert(0, p)


_ensure_path()

import ml_dtypes  # noqa: E402

import concourse.bacc as bacc  # noqa: E402
import concourse.bass as bass  # noqa: E402
import concourse.mybir as mybir  # noqa: E402
import concourse.tile as tile  # noqa: E402
from concourse import bass_utils  # noqa: E402

F32 = mybir.dt.float32
I16 = mybir.dt.int16
F8 = mybir.dt.float8e4
F16 = mybir.dt.float16
NP_F8 = ml_dtypes.float8_e4m3

C = 256  # channels
P = 128  # partitions
NCH = C // P  # channel chunks (2)
WID = 56  # image width (fixed)
PW = WID + 1  # padded width (57): right pad shared with next row's left pad
RB = 8  # output rows per PSUM tile
EPS = 1e-5

# module-level knobs (test.py may set these)
TRACE = False
TRACE_KW = {}

Alu = mybir.AluOpType
Act = mybir.ActivationFunctionType
DR = mybir.MatmulPerfMode.DoubleRow


def build_nc(n_img, h, n_cores):
    """Build the SPMD Bass program (same on every core)."""
    assert h % RB == 0 and h % 4 == 0
    nrb = h // RB
    ph = h + 2
    plane = ph * PW
    pstride = (plane + 15) // 16 * 16  # DoubleRow needs 16B-aligned k-step
    hw = h * WID
    FREE = RB * PW  # matmul free dim (456); col w=56 of each row is junk
    hrows = h // 2
    half = hrows * WID
    qrows = h // 4
    qsz = qrows * WID
    m_loc = n_img * hw
    m_glob = n_cores * m_loc

    nc = bacc.Bacc(
        "TRN2", target_bir_lowering=False, debug=False, num_devices=n_cores
    )
    x_d = nc.dram_tensor("x", [n_img, C, h, WID], F32, kind="ExternalInput").ap()
    w_d = [
        nc.dram_tensor(f"wb{s + 1}", [P, 9, NCH, C], F8, kind="ExternalInput").ap()
        for s in range(2)
    ]
    # coefs[:, ch, k]: k=0 gamma1*scale1, 1 beta1, 2 gamma2*scale2, 3 beta2,
    #                 4 scale1^2 (bcast), 5 scale2^2 (bcast)
    cf_d = nc.dram_tensor("coefs", [P, NCH, 6], F32, kind="ExternalInput").ap()
    out_d = nc.dram_tensor("out", [n_img, C, h, WID], F32, kind="ExternalOutput").ap()

    with tile.TileContext(nc) as tc:
        with (
            tc.tile_pool(name="persist", bufs=1) as persist,
            tc.tile_pool(name="abuf", bufs=1) as abuf,
            tc.tile_pool(name="cbuf", bufs=1) as cbuf,
            tc.tile_pool(name="statsp", bufs=1) as statsp,
            tc.tile_pool(name="xs", bufs=4) as xs,
            tc.tile_pool(name="ftp", bufs=2) as ftp,
            tc.tile_pool(name="small", bufs=2) as small,
            tc.tile_pool(name="ps", bufs=8, space="PSUM") as psp,
            tc.tile_pool(name="dram", bufs=2, space="DRAM") as dramp,
        ):
            # ---- persistent tiles; boot DMAs ----
            w_t = [
                persist.tile([P, 9, NCH, C], F8, tag=f"w{s}", name=f"w{s}")
                for s in range(2)
            ]
            nc.sync.dma_start(out=w_t[0], in_=w_d[0])
            coefs = persist.tile([P, NCH, 6], F32, tag="coefs")
            nc.sync.dma_start(out=coefs, in_=cf_d)
            nc.scalar.dma_start(out=w_t[1], in_=w_d[1])  # spare queue; not urgent
            eps_t = persist.tile([P, 1], F32, tag="eps")
            nc.vector.memset(eps_t, EPS)
            stt1 = persist.tile([P, NCH, 6], F32, tag="stt1")
            stt2 = persist.tile([P, NCH, 6], F32, tag="stt2")
            b12 = persist.tile([P, NCH, 1], F32, tag="b12")

            # prewarm ACT tables (Sign now; Sqrt/Identity used post-collective)
            warm = small.tile([P, 1], F32, tag="warm")
            nc.scalar.activation(out=warm, in_=eps_t, func=Act.Sign)
            nc.scalar.activation(out=warm, in_=eps_t, func=Act.Sqrt)
            nc.scalar.activation(out=warm, in_=eps_t, func=Act.Identity,
                                 bias=eps_t)

            c_t = [
                {
                    (i, ch): cbuf.tile(
                        [P, hw], I16, tag=f"c{s}_{i}_{ch}", name=f"c{s}_{i}_{ch}"
                    )
                    for i in range(n_img)
                    for ch in range(NCH)
                }
                for s in range(2)
            ]
            stats1 = statsp.tile([P, NCH, n_img * nrb, 6], F32, tag="stats1")
            stats2 = statsp.tile([P, NCH, n_img * nrb, 6], F32, tag="stats2")

            a1 = [
                abuf.tile([P, NCH, pstride], F8, tag=f"a1_{i}", name=f"a1_{i}")
                for i in range(n_img)
            ]
            a2 = [
                abuf.tile([P, NCH, pstride], F8, tag=f"a2_{i}", name=f"a2_{i}")
                for i in range(n_img)
            ]

            def zero_borders(eng, a_t, ch):
                """Zero pad rows/col + tail of one padded plane ([ph, PW])."""
                pl = a_t[:, ch, 0:plane].rearrange("p (r c) -> p r c", c=PW)
                eng.memset(pl[:, 0:1, :], 0.0)  # top pad row
                eng.memset(pl[:, h + 1 : h + 2, :], 0.0)  # bottom pad row
                eng.memset(pl[:, 1 : h + 1, 0:1], 0.0)  # left pad col
                if pstride > plane:
                    eng.memset(a_t[:, ch, plane:pstride], 0.0)

            def sign_view(a_t, ch):
                """Interior [P, h, WID] view of the padded plane."""
                return a_t[:, ch, 0:plane].rearrange(
                    "p (r c) -> p r c", c=PW
                )[:, 1 : h + 1, 1 : WID + 1]

            # a1[0] borders on DVE (idle at boot); everything else on GpSimd
            for ch in range(NCH):
                zero_borders(nc.vector, a1[0], ch)

            # prewarm the collective path (first collective pays ~60us setup)
            wc_in = dramp.tile([P, 1], F32, tag="wc_in", name="wc_in")
            wc_out = dramp.tile(
                [n_cores * P, 1], F32, tag="wc_out", name="wc_out",
                addr_space="Shared" if n_cores > 4 else "Local",
            )
            nc.gpsimd.dma_start(out=wc_in, in_=cf_d[:, 0, 0:1])
            nc.gpsimd.collective_compute(
                "AllGather",
                Alu.bypass,
                replica_groups=[list(range(n_cores))],
                ins=[wc_in.opt()],
                outs=[wc_out.opt()],
            )

            # ---- stage-1 x loads (halves, dual queues) + signs (quarters) ----
            for i in range(n_img):
                xts = {}
                for hh in range(2):
                    for ch in range(NCH):
                        xt = xs.tile([P, half], F32, tag="x", name=f"x1_{i}")
                        eng = nc.sync if ch == 0 else nc.gpsimd
                        eng.dma_start(
                            out=xt.rearrange("p (r c) -> p r c", c=WID),
                            in_=x_d[i, ch * P : (ch + 1) * P,
                                    hh * hrows : (hh + 1) * hrows],
                        )
                        xts[(ch, hh)] = xt
                for q in range(4):
                    for ch in range(NCH):
                        xt = xts[(ch, q // 2)]
                        sl = slice((q % 2) * qsz, (q % 2 + 1) * qsz)
                        nc.scalar.activation(
                            out=sign_view(a1[i], ch)[
                                :, q * qrows : (q + 1) * qrows, :
                            ],
                            in_=xt[:, sl].rearrange("p (r c) -> p r c", c=WID),
                            func=Act.Sign,
                        )
                # remaining borders on gpsimd, pipelined behind img i's loads
                if i + 1 < n_img:
                    for ch in range(NCH):
                        zero_borders(nc.gpsimd, a1[i + 1], ch)
                for ch in range(NCH):
                    zero_borders(nc.gpsimd, a2[i], ch)

            def conv_chunk(s, wt, a_t, i, ch_o, stats_t):
                """3x3 binary conv, one image, one out-chunk; drain + stats.

                Drains live on DVE (with bn_stats) so the ACT FIFO carries
                only signs/casts/final scale ops and never head-of-line
                blocks PSUM bank recycling.
                """
                c_tile = c_t[s][(i, ch_o)]
                for rb in range(nrb):
                    ps = psp.tile([P, FREE], F32, tag="ps", name="ps")
                    for tap in range(9):
                        dh, dw = divmod(tap, 3)
                        off = (rb * RB + dh) * PW + dw
                        nc.tensor.matmul(
                            ps,
                            wt[:, tap, :, ch_o * P : (ch_o + 1) * P],
                            a_t[:, 0:2, off : off + FREE],
                            start=(tap == 0),
                            stop=(tap == 8),
                            perf_mode=DR,
                        )
                    pv = ps.rearrange("p (r c) -> p r c", c=PW)[:, :, 0:WID]
                    cs = c_tile[:, rb * RB * WID : (rb + 1) * RB * WID]
                    csv = cs.rearrange("p (r c) -> p r c", c=WID)
                    nc.vector.tensor_copy(out=csv, in_=pv)
                    nc.vector.bn_stats(out=stats_t[:, ch_o, i * nrb + rb], in_=cs)

            def ar_launch(stats_t, ch):
                """Aggregate local stats for one chunk, launch AllGather."""
                mv = small.tile([P, 2], F32, tag="mv")
                nc.vector.bn_aggr(out=mv, in_=stats_t[:, ch])
                ar = small.tile([P, 2], F32, tag="ar")
                tmp = small.tile([P, 1], F32, tag="tmp")
                # S1 = mean * m_loc ; S2 = (var + mean^2) * m_loc
                nc.vector.tensor_scalar(
                    out=ar[:, 0:1], in0=mv[:, 0:1],
                    scalar1=float(m_loc), scalar2=None, op0=Alu.mult,
                )
                nc.vector.tensor_mul(tmp, mv[:, 0:1], mv[:, 0:1])
                nc.vector.tensor_add(tmp, tmp, mv[:, 1:2])
                nc.vector.tensor_scalar(
                    out=ar[:, 1:2], in0=tmp,
                    scalar1=float(m_loc), scalar2=None, op0=Alu.mult,
                )
                d_in = dramp.tile([P, 2], F32, tag="d_in")
                d_out = dramp.tile(
                    [n_cores * P, 2], F32, tag="d_out",
                    addr_space="Shared" if n_cores > 4 else "Local",
                )
                nc.gpsimd.dma_start(out=d_in, in_=ar)
                nc.gpsimd.collective_compute(
                    "AllGather",
                    Alu.bypass,
                    replica_groups=[list(range(n_cores))],
                    ins=[d_in.opt()],
                    outs=[d_out.opt()],
                )
                return d_out

            def ar_finish(d_out, ch, gs_col, b_col, ssq_col, stt):
                """Fetch gathered stats, compute per-channel A, B for chunk.

                stt columns: 0 mu_c, 1 var_c, 2 inv, 3 A, 4 B, 5 tmp
                """
                g_all = small.tile([P, n_cores, 2], F32, tag="g_all")
                nc.gpsimd.dma_start(
                    out=g_all, in_=d_out.rearrange("(r p) f -> p r f", p=P)
                )
                g = small.tile([P, 2], F32, tag="g")
                nc.vector.tensor_reduce(
                    out=g,
                    in_=g_all.rearrange("p r f -> p f r"),
                    axis=mybir.AxisListType.X,
                    op=Alu.add,
                )
                inv_m = float(1.0 / m_glob)
                s = stt[:, ch]
                nc.vector.tensor_scalar(
                    out=s[:, 0:1], in0=g[:, 0:1],
                    scalar1=inv_m, scalar2=None, op0=Alu.mult,
                )
                nc.vector.tensor_scalar(
                    out=s[:, 1:2], in0=g[:, 1:2],
                    scalar1=inv_m, scalar2=None, op0=Alu.mult,
                )
                nc.vector.tensor_mul(s[:, 5:6], s[:, 0:1], s[:, 0:1])
                nc.vector.tensor_sub(s[:, 1:2], s[:, 1:2], s[:, 5:6])
                # sd = sqrt(var_c * scale^2 + eps); inv = 1/sd
                nc.scalar.activation(
                    out=s[:, 2:3], in_=s[:, 1:2], func=Act.Sqrt,
                    bias=eps_t, scale=coefs[:, 0, ssq_col : ssq_col + 1],
                )
                nc.vector.reciprocal(out=s[:, 2:3], in_=s[:, 2:3])
                # A = inv * (gamma*scale);  B = beta - mu_c * A
                nc.vector.tensor_mul(
                    s[:, 3:4], s[:, 2:3], coefs[:, ch, gs_col : gs_col + 1]
                )
                nc.vector.tensor_mul(s[:, 5:6], s[:, 0:1], s[:, 3:4])
                nc.vector.tensor_sub(
                    s[:, 4:5], coefs[:, ch, b_col : b_col + 1], s[:, 5:6]
                )

            inner_t = {}

            def resign_img(i, ch):
                """Stage-2 input prep for one (image, chunk):
                t = c1*A1 + x; inner = t (fp16, reusing the c1 slot);
                a2 = Sign(t + B1)."""
                c1 = c_t[0][(i, ch)]
                xts = []
                for hh in range(2):
                    xt = xs.tile([P, half], F32, tag="x", name=f"x2_{i}_{ch}")
                    nc.sync.dma_start(
                        out=xt.rearrange("p (r c) -> p r c", c=WID),
                        in_=x_d[i, ch * P : (ch + 1) * P,
                                hh * hrows : (hh + 1) * hrows],
                    )
                    nc.vector.scalar_tensor_tensor(
                        out=xt,
                        in0=c1[:, hh * half : (hh + 1) * half],
                        scalar=stt1[:, ch, 3:4],
                        in1=xt,
                        op0=Alu.mult,
                        op1=Alu.add,
                    )
                    xts.append(xt)
                for q in range(4):
                    sl = slice((q % 2) * qsz, (q % 2 + 1) * qsz)
                    nc.scalar.activation(
                        out=sign_view(a2[i], ch)[:, q * qrows : (q + 1) * qrows, :],
                        in_=xts[q // 2][:, sl].rearrange("p (r c) -> p r c", c=WID),
                        func=Act.Sign,
                        bias=stt1[:, ch, 4:5],
                    )
                it = cbuf.tile(
                    [P, hw], F16, tag=f"c0_{i}_{ch}", name=f"inner_{i}_{ch}"
                )
                for hh in range(2):
                    nc.scalar.copy(
                        out=it[:, hh * half : (hh + 1) * half], in_=xts[hh]
                    )
                inner_t[(i, ch)] = it

            def final_img(i, ch, out_eng):
                """u = c2*A2 + (B1+B2) on ACT; out = u + t on DVE; write out."""
                c2 = c_t[1][(i, ch)]
                for hh in range(2):
                    sl = slice(hh * half, (hh + 1) * half)
                    ft = ftp.tile([P, half], F16, tag="f16", name="ft")
                    nc.scalar.activation(
                        out=ft, in_=c2[:, sl],
                        func=Act.Identity,
                        bias=b12[:, ch], scale=stt2[:, ch, 3:4],
                    )
                    fo = xs.tile([P, half], F32, tag="x", name="fout")
                    nc.vector.tensor_add(fo, ft, inner_t[(i, ch)][:, sl])
                    out_eng.dma_start(
                        out=out_d[i, ch * P : (ch + 1) * P,
                                  hh * hrows : (hh + 1) * hrows],
                        in_=fo.rearrange("p (r c) -> p r c", c=WID),
                    )

            # ================= stage 1 =================
            with nc.named_scope("conv1a"):
                for i in range(n_img):
                    conv_chunk(0, w_t[0], a1[i], i, 0, stats1)
            with nc.named_scope("ar1a"):
                d1a = ar_launch(stats1, 0)
            # conv1 chunk 1 interleaved with chunk-0 re-sign.  ar_finish (and
            # the collective-gated t-computes behind it) is emitted after two
            # images of chunk-1 drains so the DVE FIFO never head-of-line
            # blocks the PSUM drains the PE needs for bank reuse.
            with nc.named_scope("conv1b"):
                conv_chunk(0, w_t[0], a1[0], 0, 1, stats1)
                conv_chunk(0, w_t[0], a1[1], 1, 1, stats1)
                ar_finish(d1a, 0, 0, 1, 4, stt1)
                resign_img(0, 0)
                conv_chunk(0, w_t[0], a1[2], 2, 1, stats1)
                resign_img(1, 0)
                conv_chunk(0, w_t[0], a1[3], 3, 1, stats1)
                resign_img(2, 0)
                resign_img(3, 0)
            with nc.named_scope("ar1b"):
                d1b = ar_launch(stats1, 1)
                ar_finish(d1b, 1, 0, 1, 4, stt1)

            # ================= stage 2 =================
            with nc.named_scope("conv2a"):
                resign_img(0, 1)
                conv_chunk(1, w_t[1], a2[0], 0, 0, stats2)
                resign_img(1, 1)
                conv_chunk(1, w_t[1], a2[1], 1, 0, stats2)
                resign_img(2, 1)
                conv_chunk(1, w_t[1], a2[2], 2, 0, stats2)
                resign_img(3, 1)
                conv_chunk(1, w_t[1], a2[3], 3, 0, stats2)
            with nc.named_scope("ar2a"):
                d2a = ar_launch(stats2, 0)
            with nc.named_scope("conv2b"):
                conv_chunk(1, w_t[1], a2[0], 0, 1, stats2)
                conv_chunk(1, w_t[1], a2[1], 1, 1, stats2)
                ar_finish(d2a, 0, 2, 3, 5, stt2)
                nc.vector.tensor_add(
                    b12[:, 0], stt1[:, 0, 4:5], stt2[:, 0, 4:5]
                )
                final_img(0, 0, nc.sync)
                conv_chunk(1, w_t[1], a2[2], 2, 1, stats2)
                final_img(1, 0, nc.sync)
                conv_chunk(1, w_t[1], a2[3], 3, 1, stats2)
                final_img(2, 0, nc.sync)
                final_img(3, 0, nc.sync)
            with nc.named_scope("ar2b"):
                d2b = ar_launch(stats2, 1)
                ar_finish(d2b, 1, 2, 3, 5, stt2)
                nc.vector.tensor_add(
                    b12[:, 1], stt1[:, 1, 4:5], stt2[:, 1, 4:5]
                )
            with nc.named_scope("final"):
                for i in range(n_img):
                    final_img(i, 1, nc.gpsimd if i % 2 else nc.sync)
    return nc


def prep_inputs(x, W1, gamma1, beta1, W2, gamma2, beta2, n_cores, n_img):
    """Host-side prep: shard x, binarize/permute weights, pack BN coefs."""

    def prep_w(Wm):
        Wm = np.asarray(Wm, np.float32)
        scale = np.float32(np.mean(np.abs(Wm)))
        s = np.sign(Wm).astype(NP_F8)  # [co, ci, 3, 3]
        t = s.reshape(C, NCH, P, 3, 3)  # co, kch, p, dh, dw
        t = np.ascontiguousarray(t.transpose(2, 3, 4, 1, 0))  # p,dh,dw,kch,co
        return t.reshape(P, 9, NCH, C), scale

    w1b, s1 = prep_w(W1)
    w2b, s2 = prep_w(W2)
    g1 = np.asarray(gamma1, np.float32)
    b1 = np.asarray(beta1, np.float32)
    g2 = np.asarray(gamma2, np.float32)
    b2 = np.asarray(beta2, np.float32)
    coefs = np.zeros((P, NCH, 6), np.float32)
    coefs[:, :, 0] = (g1 * s1).reshape(NCH, P).T
    coefs[:, :, 1] = b1.reshape(NCH, P).T
    coefs[:, :, 2] = (g2 * s2).reshape(NCH, P).T
    coefs[:, :, 3] = b2.reshape(NCH, P).T
    coefs[:, :, 4] = np.float32(s1) ** 2
    coefs[:, :, 5] = np.float32(s2) ** 2

    x = np.asarray(x, np.float32)
    n, _, h, _ = x.shape
    assert n == n_cores * n_img
    xs = x.reshape(n_cores, n_img, C, h, WID)
    return [
        {
            "x": np.ascontiguousarray(xs[c]),
            "wb1": w1b,
            "wb2": w2b,
            "coefs": coefs,
        }
        for c in range(n_cores)
    ]


_NC_CACHE = {}


def _get_nc(n_img, h, n_cores):
    key = (n_img, h, n_cores)
    if key not in _NC_CACHE:
        nc = build_nc(n_img, h, n_cores)
        nc.compile()
        _NC_CACHE[key] = nc
    return _NC_CACHE[key]


_LAST_RESULT = None  # BassKernelResults of the most recent run (for test.py)


def kernel(x, W1, gamma1, beta1, W2, gamma2, beta2):
    global _LAST_RESULT
    x = np.asarray(x, np.float32)
    n_cores = 8
    n = x.shape[0]
    assert n % n_cores == 0
    n_img = n // n_cores
    h = x.shape[2]

    nc = _get_nc(n_img, h, n_cores)
    in_maps = prep_inputs(
        x, W1, gamma1, beta1, W2, gamma2, beta2, n_cores, n_img
    )
    res = bass_utils.run_bass_kernel_spmd(
        nc, in_maps, core_ids=list(range(n_cores)), trace=TRACE, **TRACE_KW
    )
    _LAST_RESULT = res
    out = np.concatenate([res.results[c]["out"] for c in range(n_cores)], axis=0)
    return out
